# revision 3
# baseline (speedup 1.0000x reference)
"""Multi-head attention (B=2, S=2048, D=768, H=12) on 8 TRN2 NeuronCores.

Sharding: data-parallel over batch (2) x tensor-parallel over heads
(4 groups of 3 heads), Megatron-style. Core c handles batch c//4 and
heads 3*(c%4) .. 3*(c%4)+2. Each core computes a partial (S, D) output
(its heads' contribution through Wo); the host sums the 4 partials per
batch. bo is added on exactly one core per batch (the others get zeros).

Device kernel (per core), all matmuls bf16 with fp32 PSUM accumulation:
  phase 1: qT/kT (head-dim on partitions) and v (natural layout, with a
           ones column appended for the softmax denominator) via
           projections from xT = x[b].T (host-pretransposed, bf16).
  phase 2: per 1024-wide q block, per head: scoresT = k @ qT on PE,
           exp(0.125*scores) on ACT (PSUM->SBUF, bf16), out^T = [v|1]^T
           @ attnT accumulated over 16 k-tiles in PSUM. Row 64 of the
           accumulator is the softmax denominator.
  phase 3: per 128-row output block: transpose denominators to natural
           layout with tiny K=1 matmuls, reciprocal on DVE, per-head
           output projection P_h = outT_h.T @ Wo_h, and the final
           combine acc = sum_h P_h * (1/d_h) + bo on DVE.
"""

import numpy as np
import ml_dtypes

BF16 = ml_dtypes.bfloat16

B, S, D = 2, 2048, 768
H, HD = 12, 64
HPC = 3            # heads per core
DC = HPC * HD      # 192 projection columns per core
NKT = S // 128     # 16 k-tiles
NDT = D // 128     # 6 contraction tiles for projections
QB = 1024          # q-block width for scores/exp
NQB = S // QB      # 2
NSUB = S // 128    # 16 output row blocks

_cache = {}


def _build_nc():
    import concourse.bacc as bacc
    import concourse.mybir as mybir
    import concourse.tile as tile

    f32 = mybir.dt.float32
    bf16 = mybir.dt.bfloat16
    Exp = mybir.ActivationFunctionType.Exp
    MUL = mybir.AluOpType.mult
    ADD = mybir.AluOpType.add

    nc = bacc.Bacc("TRN2", target_bir_lowering=False, debug=False, num_devices=1)

    def mm(out_ap, lhsT, rhs, start, stop, nmax=512):
        # matmul with moving free dim split into <=512 chunks
        n = rhs.shape[-1]
        for i in range(0, n, nmax):
            j = min(i + nmax, n)
            nc.tensor.matmul(out_ap[:, i:j], lhsT, rhs[:, i:j],
                             start=start, stop=stop)

    xT = nc.dram_tensor("xT", (D, S), bf16, kind="ExternalInput")
    wqkv = nc.dram_tensor("wqkv", (D, 3 * DC), bf16, kind="ExternalInput")
    wo = nc.dram_tensor("wo", (HD, HPC, D), bf16, kind="ExternalInput")
    bqk0 = nc.dram_tensor("bqk0", (128, 2), f32, kind="ExternalInput")
    bqk1 = nc.dram_tensor("bqk1", (HD, 2), f32, kind="ExternalInput")
    bv = nc.dram_tensor("bv", (1, DC), bf16, kind="ExternalInput")
    bo_t = nc.dram_tensor("bo_t", (128, D), f32, kind="ExternalInput")
    out = nc.dram_tensor("out", (S, D), f32, kind="ExternalOutput")

    with tile.TileContext(nc) as tc:
        with (
            tc.tile_pool(name="persist", bufs=1) as sbp,
            tc.tile_pool(name="att", bufs=4) as att,
            tc.tile_pool(name="accsb", bufs=3) as accsb,
            tc.tile_pool(name="scp", bufs=2, space="PSUM") as scp,
            tc.tile_pool(name="acp", bufs=2, space="PSUM") as acp,
        ):
            # ---- persistent SBUF tensors + input DMAs ----
            xT_sb = []
            wqkv_sb = []
            for kt in range(NDT):
                xt = sbp.tile([128, S], bf16, name=f"xT_sb{kt}")
                nc.sync.dma_start(out=xt, in_=xT.ap()[kt * 128:(kt + 1) * 128, :])
                xT_sb.append(xt)
                wt = sbp.tile([128, 3 * DC], bf16, name=f"wqkv_sb{kt}")
                nc.sync.dma_start(out=wt, in_=wqkv.ap()[kt * 128:(kt + 1) * 128, :])
                wqkv_sb.append(wt)
            wo_sb = sbp.tile([HD, HPC, D], bf16)
            nc.sync.dma_start(out=wo_sb, in_=wo.ap())
            bqk0_sb = sbp.tile([128, 2], f32)
            nc.sync.dma_start(out=bqk0_sb, in_=bqk0.ap())
            bqk1_sb = sbp.tile([HD, 2], f32)
            nc.sync.dma_start(out=bqk1_sb, in_=bqk1.ap())
            bv_sb = sbp.tile([1, DC], bf16)
            nc.sync.dma_start(out=bv_sb, in_=bv.ap())
            bo_sb = sbp.tile([128, D], f32)
            nc.sync.dma_start(out=bo_sb, in_=bo_t.ap())

            ones_row = sbp.tile([1, 128], bf16)
            nc.vector.memset(ones_row, 1.0)
            one1 = sbp.tile([128, 1], f32)
            nc.vector.memset(one1, 1.0)

            qT0 = sbp.tile([128, S], bf16)   # heads 0 (p0:64) / 1 (p64:128)
            kT0 = sbp.tile([128, S], bf16)
            qT1 = sbp.tile([HD, S], bf16)    # head 2
            kT1 = sbp.tile([HD, S], bf16)
            vv = [sbp.tile([128, HPC, HD + 1], bf16, name=f"vv{st}")
                  for st in range(NKT)]
            outT = sbp.tile([HD, HPC, S], bf16)
            # denominators, staged on partition 64 (same partition the
            # attn@v accumulator's ones-row lands on): [h*S + q]
            stag = sbp.tile([128, HPC * S], f32)

            # ---- phase 1: projections ----
            # qT/kT: lhsT = W cols chunk, rhs = xT  -> psum (M, QB)
            for qb in range(NQB):
                qs = slice(qb * QB, (qb + 1) * QB)
                for name, col0, m, dest, bias_sb, bcol in (
                    ("qc0", 0, 128, qT0, bqk0_sb, 0),
                    ("kc0", DC, 128, kT0, bqk0_sb, 1),
                    ("qc1", 128, HD, qT1, bqk1_sb, 0),
                    ("kc1", DC + 128, HD, kT1, bqk1_sb, 1),
                ):
                    ps = scp.tile([m, QB], f32, name="pj", tag="sc")
                    for kt in range(NDT):
                        mm(ps, wqkv_sb[kt][:, col0:col0 + m],
                           xT_sb[kt][:, qs],
                           start=(kt == 0), stop=(kt == NDT - 1))
                    nc.vector.tensor_scalar_add(
                        dest[:, qs], ps, bias_sb[:m, bcol:bcol + 1])

            # v natural layout + ones column
            for st in range(NKT):
                ss = slice(st * 128, (st + 1) * 128)
                vps = scp.tile([128, DC], f32, name="vps", tag="sc")
                for kt in range(NDT):
                    nc.tensor.matmul(
                        vps, xT_sb[kt][:, ss], wqkv_sb[kt][:, 2 * DC:3 * DC],
                        start=(kt == 0), stop=False)
                nc.tensor.matmul(vps, ones_row, bv_sb, start=False, stop=True)
                nc.vector.tensor_copy(
                    vv[st][:, :, 0:HD],
                    vps.rearrange("p (h d) -> p h d", h=HPC))
                nc.vector.memset(vv[st][:, :, HD:HD + 1], 1.0)

            # ---- phase 2: attention ----
            def head_slices(h):
                if h < 2:
                    return (qT0[h * HD:(h + 1) * HD, :],
                            kT0[h * HD:(h + 1) * HD, :])
                return qT1, kT1

            def attn_pass(qb, heads):
                qs = slice(qb * QB, (qb + 1) * QB)
                accs = {h: acp.tile([HD + 1, QB], f32, name=f"acc{h}", tag="ac")
                        for h in heads}
                sc_t = {}
                at_t = {}

                def emit_scores(kt):
                    for h in heads:
                        qh, kh = head_slices(h)
                        sc = scp.tile([128, QB], f32, name="sc", tag="sc")
                        mm(sc, kh[:, kt * 128:(kt + 1) * 128], qh[:, qs],
                           start=True, stop=True)
                        sc_t[h] = sc

                emit_scores(0)
                for kt in range(NKT):
                    for h in heads:
                        at = att.tile([128, QB], bf16, name="at", tag="at")
                        nc.scalar.activation(at, sc_t[h], Exp, scale=0.125)
                        at_t[h] = at
                    if kt + 1 < NKT:
                        emit_scores(kt + 1)
                    for h in heads:
                        mm(accs[h], vv[kt][:, h, :], at_t[h],
                           start=(kt == 0), stop=(kt == NKT - 1))
                for h in heads:
                    nc.vector.tensor_copy(outT[:, h, qs], accs[h][0:HD, :])
                    nc.vector.tensor_copy(
                        stag[64:65, h * S + qb * QB: h * S + (qb + 1) * QB],
                        accs[h][HD:HD + 1, :])

            for qb in range(NQB):
                attn_pass(qb, (0, 1))
                attn_pass(qb, (2,))

            # ---- phase 3: denominator transpose + Wo + combine ----
            for sub in range(NSUB):
                rs = slice(sub * 128, (sub + 1) * 128)
                dT = scp.tile([128, 4], f32, name="dT", tag="sc")
                for h in range(HPC):
                    nc.tensor.matmul(
                        dT[:, h:h + 1],
                        stag[64:65, h * S + sub * 128: h * S + (sub + 1) * 128],
                        one1[64:65, 0:1],
                        start=True, stop=True)
                dr = accsb.tile([128, 4], f32, name="dr", tag="dr")
                nc.vector.reciprocal(dr[:, 0:HPC], dT[:, 0:HPC])
                acc = accsb.tile([128, D], f32, name="acc", tag="acc")
                for h in range(HPC):
                    P = scp.tile([128, D], f32, name="P", tag="sc")
                    nc.tensor.matmul(P[:, 0:512], outT[:, h, rs],
                                     wo_sb[:, h, 0:512], start=True, stop=True)
                    nc.tensor.matmul(P[:, 512:D], outT[:, h, rs],
                                     wo_sb[:, h, 512:D], start=True, stop=True)
                    nc.vector.scalar_tensor_tensor(
                        out=acc, in0=P, scalar=dr[:, h:h + 1],
                        in1=(bo_sb if h == 0 else acc), op0=MUL, op1=ADD)
                nc.sync.dma_start(out=out.ap()[rs, :], in_=acc)

    nc.compile()
    return nc


def _prep_core_inputs(x, Wq, bq, Wk, bk, Wv, bv, Wo, bo, core):
    b, g = divmod(core, 4)
    cs = slice(g * DC, (g + 1) * DC)
    xTb = np.ascontiguousarray(x[b].T).astype(BF16)
    wqkv = np.concatenate([Wq[:, cs], Wk[:, cs], Wv[:, cs]], axis=1).astype(BF16)
    wo_c = Wo[cs, :].reshape(HPC, HD, D).transpose(1, 0, 2)  # (HD, HPC, D)
    bq_c, bk_c = bq[cs], bk[cs]
    bqk0 = np.stack([bq_c[:128], bk_c[:128]], axis=1).astype(np.float32)
    bqk1 = np.stack([bq_c[128:], bk_c[128:]], axis=1).astype(np.float32)
    bo_t = (np.broadcast_to(bo, (128, D)) if g == 0
            else np.zeros((128, D), np.float32))
    return {
        "xT": xTb,
        "wqkv": np.ascontiguousarray(wqkv),
        "wo": np.ascontiguousarray(wo_c).astype(BF16),
        "bqk0": np.ascontiguousarray(bqk0),
        "bqk1": np.ascontiguousarray(bqk1),
        "bv": np.ascontiguousarray(bv[cs]).reshape(1, DC).astype(BF16),
        "bo_t": np.ascontiguousarray(bo_t).astype(np.float32),
    }


def kernel(x, Wq, bq, Wk, bk, Wv, bv, Wo, bo, _trace=False):
    from concourse.bass_utils import run_bass_kernel_spmd

    x = np.asarray(x, np.float32)
    Wq, bq = np.asarray(Wq, np.float32), np.asarray(bq, np.float32)
    Wk, bk = np.asarray(Wk, np.float32), np.asarray(bk, np.float32)
    Wv, bv = np.asarray(Wv, np.float32), np.asarray(bv, np.float32)
    Wo, bo = np.asarray(Wo, np.float32), np.asarray(bo, np.float32)

    if "nc" not in _cache:
        _cache["nc"] = _build_nc()
    nc = _cache["nc"]

    in_maps = [_prep_core_inputs(x, Wq, bq, Wk, bk, Wv, bv, Wo, bo, c)
               for c in range(8)]
    res = run_bass_kernel_spmd(nc, in_maps, core_ids=list(range(8)),
                               trace=_trace)
    _cache["last_result"] = res
    parts = [r["out"] for r in res.results]
    full = np.zeros((B, S, D), np.float32)
    for b in range(B):
        full[b] = parts[4 * b] + parts[4 * b + 1] + parts[4 * b + 2] + parts[4 * b + 3]
    return full


# revision 4
# speedup vs baseline: 1.2338x; 1.2338x over previous
"""Multi-head attention (B=2, S=2048, D=768, H=12) on 8 TRN2 NeuronCores.

Sharding: data-parallel over batch (2) x tensor-parallel over heads
(4 groups of 3 heads), Megatron-style. Core c handles batch c//4 and
heads 3*(c%4) .. 3*(c%4)+2. Each core computes a partial (S, D) output
(its heads' contribution through Wo); the host sums the 4 partials per
batch. bo is added on exactly one core per batch (the others get zeros).

Device kernel (per core), all matmuls bf16 with fp32 PSUM accumulation,
every matmul padded to M=128 output partitions (keeps FWL + PE activity
monitor engaged):
  phase 1: qT/kT (head-dim on partitions, zero-padded chunks for head 2)
           and v (natural layout, ones column at 64, zero-padded to 128
           for the softmax denominator) projected from xT = x[b].T.
  phase 2: per 1024-wide q block, per head: scoresT = k @ qT on PE
           (h0/h1 row-packed via K=64 tile positions), exp(scores/8) on
           ACT (PSUM->SBUF, bf16), outT = [v|1|0]^T @ attnT accumulated
           over 16 k-tiles in PSUM; row 64 of the accumulator is the
           softmax denominator. Denominators go: DVE copy (partition 64)
           -> SBUF->SBUF DMA to partition 0 -> DVE reciprocal -> GPSIMD
           partition_broadcast to a (64, 1024) tile -> the outT copy is
           a fused normalize (tensor_tensor mult).
  phase 3: per 128-row output block: P = sum_h outT_h.T @ Wo_h in one
           PSUM accumulation group, one DVE add of bo, DMA out. Shares
           the accumulator PSUM slots so it overlaps the next q block.
"""

import numpy as np
import ml_dtypes

BF16 = ml_dtypes.bfloat16

B, S, D = 2, 2048, 768
H, HD = 12, 64
HPC = 3            # heads per core
DC = HPC * HD      # 192 projection columns per core
NKT = S // 128     # 16 k-tiles
NDT = D // 128     # 6 contraction tiles for projections
QB = 1024          # q-block width for scores/exp
NQB = S // QB      # 2

_cache = {}


def _build_nc():
    import concourse.bacc as bacc
    import concourse.mybir as mybir
    import concourse.tile as tile

    f32 = mybir.dt.float32
    bf16 = mybir.dt.bfloat16
    Exp = mybir.ActivationFunctionType.Exp

    nc = bacc.Bacc("TRN2", target_bir_lowering=False, debug=False, num_devices=1)

    def mm(out_ap, lhsT, rhs, start, stop, nmax=512):
        n = rhs.shape[-1]
        for i in range(0, n, nmax):
            j = min(i + nmax, n)
            nc.tensor.matmul(out_ap[:, i:j], lhsT, rhs[:, i:j],
                             start=start, stop=stop)

    # wqkv columns: [q01 | q2+pad | k01 | k2+pad | v]
    xT = nc.dram_tensor("xT", (D, S), bf16, kind="ExternalInput")
    wqkv = nc.dram_tensor("wqkv", (D, 4 * 128 + DC), bf16, kind="ExternalInput")
    wo = nc.dram_tensor("wo", (HD, HPC, D), bf16, kind="ExternalInput")
    bqk0 = nc.dram_tensor("bqk0", (128, 2), f32, kind="ExternalInput")
    bqk1 = nc.dram_tensor("bqk1", (HD, 2), f32, kind="ExternalInput")
    bv = nc.dram_tensor("bv", (1, DC), bf16, kind="ExternalInput")
    bo_t = nc.dram_tensor("bo_t", (128, D), f32, kind="ExternalInput")
    out = nc.dram_tensor("out", (S, D), f32, kind="ExternalOutput")

    with tile.TileContext(nc) as tc:
        with (
            tc.tile_pool(name="persist", bufs=1) as sbp,
            tc.tile_pool(name="att", bufs=4) as att,
            tc.tile_pool(name="stagp", bufs=2) as stagp,
            tc.tile_pool(name="dbcp", bufs=3) as dbcp,
            tc.tile_pool(name="accsb", bufs=3) as accsb,
            tc.tile_pool(name="scp", bufs=2, space="PSUM") as scp,
            tc.tile_pool(name="acp", bufs=2, space="PSUM") as acp,
        ):
            # ---- persistent SBUF tensors + input DMAs ----
            xT_sb = []
            wqkv_sb = []
            for kt in range(NDT):
                xt = sbp.tile([128, S], bf16, name=f"xT_sb{kt}")
                nc.sync.dma_start(out=xt, in_=xT.ap()[kt * 128:(kt + 1) * 128, :])
                xT_sb.append(xt)
                wt = sbp.tile([128, 4 * 128 + DC], bf16, name=f"wqkv_sb{kt}")
                nc.sync.dma_start(out=wt, in_=wqkv.ap()[kt * 128:(kt + 1) * 128, :])
                wqkv_sb.append(wt)
            wo_sb = sbp.tile([HD, HPC, D], bf16)
            nc.sync.dma_start(out=wo_sb, in_=wo.ap())
            bqk0_sb = sbp.tile([128, 2], f32)
            nc.sync.dma_start(out=bqk0_sb, in_=bqk0.ap())
            bqk1_sb = sbp.tile([HD, 2], f32)
            nc.sync.dma_start(out=bqk1_sb, in_=bqk1.ap())
            bv_sb = sbp.tile([1, DC], bf16)
            nc.sync.dma_start(out=bv_sb, in_=bv.ap())
            bo_sb = sbp.tile([128, D], f32)
            nc.sync.dma_start(out=bo_sb, in_=bo_t.ap())

            ones_row = sbp.tile([1, 128], bf16)
            nc.vector.memset(ones_row, 1.0)

            # warm up the ACT exp table early (overlaps the input DMAs)
            wu = sbp.tile([1, 8], f32)
            nc.vector.memset(wu, 0.0)
            wu2 = sbp.tile([1, 8], f32)
            nc.scalar.activation(wu2, wu, Exp, scale=1.0)

            qT0 = sbp.tile([128, S], bf16)   # heads 0 (p0:64) / 1 (p64:128)
            kT0 = sbp.tile([128, S], bf16)
            qT1 = sbp.tile([HD, S], bf16)    # head 2
            kT1 = sbp.tile([HD, S], bf16)
            # v natural: [v | ones | zeros] -> M=128
            vv = [sbp.tile([128, HPC, 128], bf16, name=f"vv{st}")
                  for st in range(NKT)]
            outT = sbp.tile([HD, HPC, S], bf16)
            drow = sbp.tile([1, HPC * S], f32)
            drec = sbp.tile([1, HPC * S], f32)

            # ---- phase 1: projections ----
            proj = (
                ("kc0", 256, 128, kT0, bqk0_sb, 1),
                ("kc1", 384, HD, kT1, bqk1_sb, 1),
                ("qc0", 0, 128, qT0, bqk0_sb, 0),
                ("qc1", 128, HD, qT1, bqk1_sb, 0),
            )

            def qkproj(qb, items):
                qs = slice(qb * QB, (qb + 1) * QB)
                for name, col0, m, dest, bias_sb, bcol in items:
                    ps = scp.tile([128, QB], f32, name="pj", tag="sc")
                    for kt in range(NDT):
                        mm(ps, wqkv_sb[kt][:, col0:col0 + 128],
                           xT_sb[kt][:, qs],
                           start=(kt == 0), stop=(kt == NDT - 1))
                    nc.vector.tensor_scalar_add(
                        dest[:, qs], ps[:m, :], bias_sb[:m, bcol:bcol + 1])

            for qb in range(NQB):
                qkproj(qb, proj[:2])      # k first: attention needs all of k
            for st in range(NKT):
                ss = slice(st * 128, (st + 1) * 128)
                vps = scp.tile([128, DC], f32, name="vps", tag="sc")
                for kt in range(NDT):
                    nc.tensor.matmul(
                        vps, xT_sb[kt][:, ss], wqkv_sb[kt][:, 512:512 + DC],
                        start=(kt == 0), stop=False)
                nc.tensor.matmul(vps, ones_row, bv_sb, start=False, stop=True)
                nc.vector.tensor_copy(
                    vv[st][:, :, 0:HD],
                    vps.rearrange("p (h d) -> p h d", h=HPC))
                nc.vector.memset(vv[st][:, :, HD:HD + 1], 1.0)
                nc.vector.memset(vv[st][:, :, HD + 1:128], 0.0)

            # ---- phase 2 + 3 ----
            def head_slices(h):
                if h < 2:
                    return (qT0[h * HD:(h + 1) * HD, :],
                            kT0[h * HD:(h + 1) * HD, :])
                return qT1, kT1

            def attn_pass(qb, heads):
                qs = slice(qb * QB, (qb + 1) * QB)
                accs = {h: acp.tile([128, QB], f32, name=f"acc{h}", tag="ac")
                        for h in heads}
                sc_t = {}
                at_t = {}

                def emit_scores(kt):
                    for h in heads:
                        qh, kh = head_slices(h)
                        sc = scp.tile([128, QB], f32, name="sc", tag="sc")
                        mm(sc, kh[:, kt * 128:(kt + 1) * 128], qh[:, qs],
                           start=True, stop=True)
                        sc_t[h] = sc

                emit_scores(0)
                for kt in range(NKT):
                    for h in heads:
                        at = att.tile([128, QB], bf16, name="at", tag="at")
                        nc.scalar.activation(at, sc_t[h], Exp, scale=0.125)
                        at_t[h] = at
                    if kt + 1 < NKT:
                        emit_scores(kt + 1)
                    for h in heads:
                        mm(accs[h], vv[kt][:, h, :], at_t[h],
                           start=(kt == 0), stop=(kt == NKT - 1))
                for h in heads:
                    acc = accs[h]
                    off = h * S + qb * QB
                    # denominator: psum row 64 -> partition 64 staging ->
                    # DMA to partition 0 -> reciprocal -> broadcast
                    stg = stagp.tile([128, QB], f32, name="stg", tag="stg")
                    nc.vector.tensor_copy(stg[64:65, :], acc[64:65, :])
                    nc.sync.dma_start(out=drow[0:1, off:off + QB],
                                      in_=stg[64:65, :])
                    nc.vector.reciprocal(drec[0:1, off:off + QB],
                                         drow[0:1, off:off + QB])
                    dbc = dbcp.tile([HD, QB], f32, name="dbc", tag="dbc")
                    nc.gpsimd.partition_broadcast(
                        dbc, drec[0:1, off:off + QB], channels=HD)
                    # normalized outT copy (bf16)
                    nc.vector.tensor_mul(outT[:, h, qs], acc[0:HD, :], dbc)

            def out_block(sub):
                rs = slice(sub * 128, (sub + 1) * 128)
                P = acp.tile([128, D], f32, name="P", tag="ac")
                for h in range(HPC):
                    mm(P, outT[:, h, rs], wo_sb[:, h, :],
                       start=(h == 0), stop=(h == HPC - 1))
                acc_sb = accsb.tile([128, D], f32, name="acc_sb", tag="accsb")
                nc.vector.tensor_add(acc_sb, P, bo_sb)
                nc.sync.dma_start(out=out.ap()[rs, :], in_=acc_sb)

            for qb in range(NQB):
                qkproj(qb, proj[2:])      # q projection for this block
                attn_pass(qb, (0, 1))
                attn_pass(qb, (2,))
                for sub in range(qb * QB // 128, (qb + 1) * QB // 128):
                    out_block(sub)

    nc.compile()
    return nc


def _prep_core_inputs(x, Wq, bq, Wk, bk, Wv, bv, Wo, bo, core):
    b, g = divmod(core, 4)
    cs = slice(g * DC, (g + 1) * DC)
    xTb = np.ascontiguousarray(x[b].T).astype(BF16)
    z64 = np.zeros((D, 64), np.float32)
    Wq_c, Wk_c, Wv_c = Wq[:, cs], Wk[:, cs], Wv[:, cs]
    wqkv = np.concatenate(
        [Wq_c[:, :128], Wq_c[:, 128:], z64,
         Wk_c[:, :128], Wk_c[:, 128:], z64, Wv_c], axis=1).astype(BF16)
    wo_c = Wo[cs, :].reshape(HPC, HD, D).transpose(1, 0, 2)  # (HD, HPC, D)
    bq_c, bk_c = bq[cs], bk[cs]
    bqk0 = np.stack([bq_c[:128], bk_c[:128]], axis=1).astype(np.float32)
    bqk1 = np.stack([bq_c[128:], bk_c[128:]], axis=1).astype(np.float32)
    bo_t = (np.broadcast_to(bo, (128, D)) if g == 0
            else np.zeros((128, D), np.float32))
    return {
        "xT": xTb,
        "wqkv": np.ascontiguousarray(wqkv),
        "wo": np.ascontiguousarray(wo_c).astype(BF16),
        "bqk0": np.ascontiguousarray(bqk0),
        "bqk1": np.ascontiguousarray(bqk1),
        "bv": np.ascontiguousarray(bv[cs]).reshape(1, DC).astype(BF16),
        "bo_t": np.ascontiguousarray(bo_t).astype(np.float32),
    }


def kernel(x, Wq, bq, Wk, bk, Wv, bv, Wo, bo, _trace=False):
    from concourse.bass_utils import run_bass_kernel_spmd

    x = np.asarray(x, np.float32)
    Wq, bq = np.asarray(Wq, np.float32), np.asarray(bq, np.float32)
    Wk, bk = np.asarray(Wk, np.float32), np.asarray(bk, np.float32)
    Wv, bv = np.asarray(Wv, np.float32), np.asarray(bv, np.float32)
    Wo, bo = np.asarray(Wo, np.float32), np.asarray(bo, np.float32)

    if "nc" not in _cache:
        _cache["nc"] = _build_nc()
    nc = _cache["nc"]

    in_maps = [_prep_core_inputs(x, Wq, bq, Wk, bk, Wv, bv, Wo, bo, c)
               for c in range(8)]
    res = run_bass_kernel_spmd(nc, in_maps, core_ids=list(range(8)),
                               trace=_trace)
    _cache["last_result"] = res
    parts = [r["out"] for r in res.results]
    full = np.zeros((B, S, D), np.float32)
    for b in range(B):
        full[b] = parts[4 * b] + parts[4 * b + 1] + parts[4 * b + 2] + parts[4 * b + 3]
    return full


# revision 5
# speedup vs baseline: 1.4208x; 1.1516x over previous
"""Multi-head attention (B=2, S=2048, D=768, H=12) on 8 TRN2 NeuronCores.

Sharding: data-parallel over batch (2) x tensor-parallel over heads
(4 groups of 3 heads), Megatron-style. Core c handles batch c//4 and
heads 3*(c%4) .. 3*(c%4)+2. Each core computes a partial (S, D) output
(its heads' contribution through Wo); the host sums the 4 partials per
batch. bo is added on exactly one core per batch (the others get zeros).

Device kernel (per core), all matmuls bf16 with fp32 PSUM accumulation,
every matmul padded to M=128 output partitions (keeps FWL + PE activity
monitor engaged):
  phase 1: qT/kT (head-dim on partitions, zero-padded chunks for head 2)
           and v (natural layout, ones column at 64, zero-padded to 128
           for the softmax denominator) projected from xT = x[b].T.
  phase 2: per 1024-wide q block, per head: scoresT = k @ qT on PE
           (h0/h1 row-packed via K=64 tile positions), exp(scores/8) on
           ACT (PSUM->SBUF, bf16), outT = [v|1|0]^T @ attnT accumulated
           over 16 k-tiles in PSUM; row 64 of the accumulator is the
           softmax denominator. Denominators go: DVE copy (partition 64)
           -> SBUF->SBUF DMA to partition 0 -> DVE reciprocal -> GPSIMD
           partition_broadcast to a (64, 1024) tile -> the outT copy is
           a fused normalize (tensor_tensor mult).
  phase 3: per 128-row output block: P = sum_h outT_h.T @ Wo_h in one
           PSUM accumulation group, one DVE add of bo, DMA out. Shares
           the accumulator PSUM slots so it overlaps the next q block.
"""

import numpy as np
import ml_dtypes

BF16 = ml_dtypes.bfloat16

B, S, D = 2, 2048, 768
H, HD = 12, 64
HPC = 3            # heads per core
DC = HPC * HD      # 192 projection columns per core
NKT = S // 128     # 16 k-tiles
NDT = D // 128     # 6 contraction tiles for projections
QB = 1024          # q-block width for scores/exp
NQB = S // QB      # 2

_cache = {}


def _build_nc():
    import concourse.bacc as bacc
    import concourse.mybir as mybir
    import concourse.tile as tile

    f32 = mybir.dt.float32
    bf16 = mybir.dt.bfloat16
    Exp = mybir.ActivationFunctionType.Exp

    nc = bacc.Bacc("TRN2", target_bir_lowering=False, debug=False, num_devices=1)

    def mm(out_ap, lhsT, rhs, start, stop, nmax=512):
        n = rhs.shape[-1]
        for i in range(0, n, nmax):
            j = min(i + nmax, n)
            nc.tensor.matmul(out_ap[:, i:j], lhsT, rhs[:, i:j],
                             start=start, stop=stop)

    # wqkv columns: [q01 | q2+pad | k01 | k2+pad | v]
    xT = nc.dram_tensor("xT", (D, S), bf16, kind="ExternalInput")
    wqkv = nc.dram_tensor("wqkv", (D, 4 * 128 + DC), bf16, kind="ExternalInput")
    wo = nc.dram_tensor("wo", (HD, HPC, D), bf16, kind="ExternalInput")
    bqk0 = nc.dram_tensor("bqk0", (128, 2), f32, kind="ExternalInput")
    bqk1 = nc.dram_tensor("bqk1", (HD, 2), f32, kind="ExternalInput")
    bv = nc.dram_tensor("bv", (1, DC), bf16, kind="ExternalInput")
    bo_t = nc.dram_tensor("bo_t", (128, D), f32, kind="ExternalInput")
    out = nc.dram_tensor("out", (S, D), f32, kind="ExternalOutput")

    with tile.TileContext(nc) as tc:
        with (
            tc.tile_pool(name="persist", bufs=1) as sbp,
            tc.tile_pool(name="att", bufs=4) as att,
            tc.tile_pool(name="stagp", bufs=2) as stagp,
            tc.tile_pool(name="dbcp", bufs=3) as dbcp,
            tc.tile_pool(name="orwp", bufs=4) as orwp,
            tc.tile_pool(name="accsb", bufs=3) as accsb,
            tc.tile_pool(name="scp", bufs=2, space="PSUM") as scp,
            tc.tile_pool(name="acp", bufs=2, space="PSUM") as acp,
        ):
            # ---- persistent SBUF tensors + input DMAs ----
            xT_sb = []
            wqkv_sb = []
            for kt in range(NDT):
                xt = sbp.tile([128, S], bf16, name=f"xT_sb{kt}")
                nc.sync.dma_start(out=xt, in_=xT.ap()[kt * 128:(kt + 1) * 128, :])
                xT_sb.append(xt)
                wt = sbp.tile([128, 4 * 128 + DC], bf16, name=f"wqkv_sb{kt}")
                nc.sync.dma_start(out=wt, in_=wqkv.ap()[kt * 128:(kt + 1) * 128, :])
                wqkv_sb.append(wt)
            wo_sb = sbp.tile([HD, HPC, D], bf16)
            nc.sync.dma_start(out=wo_sb, in_=wo.ap())
            bqk0_sb = sbp.tile([128, 2], f32)
            nc.sync.dma_start(out=bqk0_sb, in_=bqk0.ap())
            bqk1_sb = sbp.tile([HD, 2], f32)
            nc.sync.dma_start(out=bqk1_sb, in_=bqk1.ap())
            bv_sb = sbp.tile([1, DC], bf16)
            nc.sync.dma_start(out=bv_sb, in_=bv.ap())
            bo_sb = sbp.tile([128, D], f32)
            nc.sync.dma_start(out=bo_sb, in_=bo_t.ap())

            ones_row = sbp.tile([1, 128], bf16)
            nc.vector.memset(ones_row, 1.0)

            # warm up the ACT exp table early (overlaps the input DMAs)
            wu = sbp.tile([1, 8], f32)
            nc.vector.memset(wu, 0.0)
            wu2 = sbp.tile([1, 8], f32)
            nc.scalar.activation(wu2, wu, Exp, scale=1.0)

            qT0 = sbp.tile([128, S], bf16)   # heads 0 (p0:64) / 1 (p64:128)
            kT0 = sbp.tile([128, S], bf16)
            qT1 = sbp.tile([HD, S], bf16)    # head 2
            kT1 = sbp.tile([HD, S], bf16)
            # v natural: [v | ones | zeros] -> M=128
            vv = [sbp.tile([128, HPC, 128], bf16, name=f"vv{st}")
                  for st in range(NKT)]
            outT = sbp.tile([HD, HPC, S], bf16)
            drow = sbp.tile([1, HPC * S], f32)
            drec = sbp.tile([1, HPC * S], f32)

            # ---- phase 1: projections ----
            proj = (
                ("kc0", 256, 128, kT0, bqk0_sb, 1),
                ("kc1", 384, HD, kT1, bqk1_sb, 1),
                ("qc0", 0, 128, qT0, bqk0_sb, 0),
                ("qc1", 128, HD, qT1, bqk1_sb, 0),
            )

            def qkproj(qb, items):
                qs = slice(qb * QB, (qb + 1) * QB)
                for name, col0, m, dest, bias_sb, bcol in items:
                    ps = scp.tile([128, QB], f32, name="pj", tag="sc")
                    for kt in range(NDT):
                        mm(ps, wqkv_sb[kt][:, col0:col0 + 128],
                           xT_sb[kt][:, qs],
                           start=(kt == 0), stop=(kt == NDT - 1))
                    nc.vector.tensor_scalar_add(
                        dest[:, qs], ps[:m, :], bias_sb[:m, bcol:bcol + 1])

            for qb in range(NQB):
                qkproj(qb, proj[:2])      # k first: attention needs all of k
            qkproj(0, proj[2:])
            for st in range(NKT):
                ss = slice(st * 128, (st + 1) * 128)
                vps = scp.tile([128, DC], f32, name="vps", tag="sc")
                for kt in range(NDT):
                    nc.tensor.matmul(
                        vps, xT_sb[kt][:, ss], wqkv_sb[kt][:, 512:512 + DC],
                        start=(kt == 0), stop=False)
                nc.tensor.matmul(vps, ones_row, bv_sb, start=False, stop=True)
                nc.vector.tensor_copy(
                    vv[st][:, :, 0:HD],
                    vps.rearrange("p (h d) -> p h d", h=HPC))
                nc.vector.memset(vv[st][:, :, HD:HD + 1], 1.0)
                nc.vector.memset(vv[st][:, :, HD + 1:128], 0.0)

            # ---- phase 2 + 3 ----
            def head_slices(h):
                if h < 2:
                    return (qT0[h * HD:(h + 1) * HD, :],
                            kT0[h * HD:(h + 1) * HD, :])
                return qT1, kT1

            def attn_pass(qb, heads):
                qs = slice(qb * QB, (qb + 1) * QB)
                accs = {h: acp.tile([128, QB], f32, name=f"acc{h}", tag="ac")
                        for h in heads}
                sc_t = {}
                at_t = {}

                def emit_scores(kt):
                    for h in heads:
                        qh, kh = head_slices(h)
                        sc = scp.tile([128, QB], f32, name="sc", tag="sc")
                        mm(sc, kh[:, kt * 128:(kt + 1) * 128], qh[:, qs],
                           start=True, stop=True)
                        sc_t[h] = sc

                emit_scores(0)
                for kt in range(NKT):
                    for h in heads:
                        at = att.tile([128, QB], bf16, name="at", tag="at")
                        nc.scalar.activation(at, sc_t[h], Exp, scale=0.125)
                        at_t[h] = at
                    if kt + 1 < NKT:
                        emit_scores(kt + 1)
                    for h in heads:
                        mm(accs[h], vv[kt][:, h, :], at_t[h],
                           start=(kt == 0), stop=(kt == NKT - 1))
                for h in heads:
                    acc = accs[h]
                    off = h * S + qb * QB
                    # plain copies release the accumulator slot quickly
                    orw = orwp.tile([HD, QB], bf16, name="orw", tag="orw")
                    nc.vector.tensor_copy(orw, acc[0:HD, :])
                    stg = stagp.tile([128, QB], f32, name="stg", tag="stg")
                    nc.vector.tensor_copy(stg[64:65, :], acc[64:65, :])
                    # denominator: partition 64 -> DMA to partition 0 ->
                    # fast reciprocal -> broadcast over 64 partitions
                    nc.sync.dma_start(out=drow[0:1, off:off + QB],
                                      in_=stg[64:65, :])
                    nc.vector.reciprocal_approx_fast(
                        drec[0:1, off:off + QB], drow[0:1, off:off + QB])
                    dbc = dbcp.tile([HD, QB], f32, name="dbc", tag="dbc")
                    nc.gpsimd.partition_broadcast(
                        dbc, drec[0:1, off:off + QB], channels=HD)
                    # normalized outT (bf16)
                    nc.vector.tensor_mul(outT[:, h, qs], orw, dbc)

            def out_block(sub):
                rs = slice(sub * 128, (sub + 1) * 128)
                P = acp.tile([128, D], f32, name="P", tag="ac")
                for h in range(HPC):
                    mm(P, outT[:, h, rs], wo_sb[:, h, :],
                       start=(h == 0), stop=(h == HPC - 1))
                acc_sb = accsb.tile([128, D], f32, name="acc_sb", tag="accsb")
                nc.vector.tensor_add(acc_sb, P, bo_sb)
                nc.sync.dma_start(out=out.ap()[rs, :], in_=acc_sb)

            for qb in range(NQB):
                if qb > 0:
                    qkproj(qb, proj[2:])
                with tc.high_priority():
                    attn_pass(qb, (0, 1))
                    attn_pass(qb, (2,))
                for sub in range(qb * QB // 128, (qb + 1) * QB // 128):
                    out_block(sub)

    nc.compile()
    return nc


def _prep_core_inputs(x, Wq, bq, Wk, bk, Wv, bv, Wo, bo, core):
    b, g = divmod(core, 4)
    cs = slice(g * DC, (g + 1) * DC)
    xTb = np.ascontiguousarray(x[b].T).astype(BF16)
    z64 = np.zeros((D, 64), np.float32)
    Wq_c, Wk_c, Wv_c = Wq[:, cs], Wk[:, cs], Wv[:, cs]
    wqkv = np.concatenate(
        [Wq_c[:, :128], Wq_c[:, 128:], z64,
         Wk_c[:, :128], Wk_c[:, 128:], z64, Wv_c], axis=1).astype(BF16)
    wo_c = Wo[cs, :].reshape(HPC, HD, D).transpose(1, 0, 2)  # (HD, HPC, D)
    bq_c, bk_c = bq[cs], bk[cs]
    bqk0 = np.stack([bq_c[:128], bk_c[:128]], axis=1).astype(np.float32)
    bqk1 = np.stack([bq_c[128:], bk_c[128:]], axis=1).astype(np.float32)
    bo_t = (np.broadcast_to(bo, (128, D)) if g == 0
            else np.zeros((128, D), np.float32))
    return {
        "xT": xTb,
        "wqkv": np.ascontiguousarray(wqkv),
        "wo": np.ascontiguousarray(wo_c).astype(BF16),
        "bqk0": np.ascontiguousarray(bqk0),
        "bqk1": np.ascontiguousarray(bqk1),
        "bv": np.ascontiguousarray(bv[cs]).reshape(1, DC).astype(BF16),
        "bo_t": np.ascontiguousarray(bo_t).astype(np.float32),
    }


def kernel(x, Wq, bq, Wk, bk, Wv, bv, Wo, bo, _trace=False):
    from concourse.bass_utils import run_bass_kernel_spmd

    x = np.asarray(x, np.float32)
    Wq, bq = np.asarray(Wq, np.float32), np.asarray(bq, np.float32)
    Wk, bk = np.asarray(Wk, np.float32), np.asarray(bk, np.float32)
    Wv, bv = np.asarray(Wv, np.float32), np.asarray(bv, np.float32)
    Wo, bo = np.asarray(Wo, np.float32), np.asarray(bo, np.float32)

    if "nc" not in _cache:
        _cache["nc"] = _build_nc()
    nc = _cache["nc"]

    in_maps = [_prep_core_inputs(x, Wq, bq, Wk, bk, Wv, bv, Wo, bo, c)
               for c in range(8)]
    res = run_bass_kernel_spmd(nc, in_maps, core_ids=list(range(8)),
                               trace=_trace)
    _cache["last_result"] = res
    parts = [r["out"] for r in res.results]
    full = np.zeros((B, S, D), np.float32)
    for b in range(B):
        full[b] = parts[4 * b] + parts[4 * b + 1] + parts[4 * b + 2] + parts[4 * b + 3]
    return full
